# revision 7
# baseline (speedup 1.0000x reference)
"""Trainium2 Bass kernel for nn_DecSwitchedFC (MoE hard routing).

Math (per token b, expert e = y_idx[b]):
    out[b] = x[b] + z[b, e] * (relu(x[b] @ W1[e] + b1[e]) @ W2[e] + b2[e])

Strategy: expert-parallel over 8 NeuronCores, 2 experts per core, with a
fixed per-expert device capacity of 512 tokens (one expert = one token
column block).  Tokens beyond an expert's capacity (~1% of the batch for
a uniform router) are computed exactly on the host in fp32.  This gives
each core exactly 1024 device tokens with zero load imbalance across the
SPMD program.

Device data layout is fully "feature-major" (d or h on partitions, tokens
on the free axis) so no on-device transposes are needed:
    h^T[256, n]  = W1[e]^T(lhsT) @ x^T          (K=1024, 8 chunks)
    o^T[1024, n] = W2[e]^T(lhsT) @ relu(h^T+b1) (K=256, 2 chunks)

All matmuls run in bf16 (~2.4e-3 final rel err); o^T ships back in bf16
and the z-scale + residual are applied on the host in exact fp32.

Schedule notes (from perfetto traces):
  - DMA issue (~0.65us each on the issuing queue) and HBM bandwidth
    (~358 GB/s/core) are the binding constraints; total traffic is
    ~6.3 MB/core (x 2.1, W 2.1, out 2.1) ≈ 17.6us.
  - x tiles + bias stream on the Scalar (Activation DGE) queue ring while
    weights + outputs use the Sync ring, so the two input streams don't
    serialize.  (GpSimd DMA is a slow software path — 1.2us drains.)
  - The first tile is 128 tokens and only needs W1-j0 (256 KB) + x-t0
    (256 KB) before the PE can start; the last tile is 128 tokens with a
    split output DMA so the drain tail is short.
  - FC1 relu+bias runs on Scalar; the 8 FC2 bias-add copies per tile
    rotate across Vector/Scalar/GpSimd so no engine bottlenecks.
"""

import ml_dtypes
import numpy as np

import concourse.bacc as bacc
import concourse.mybir as mybir
import concourse.tile as tile
from concourse.bass_utils import run_bass_kernel_spmd

D = 1024        # model dim
H = 256         # bottleneck dim
NB = 16         # n experts
NCORES = 8
CAP = 512       # device tokens per expert (one 512-token column block)
KC1 = D // 128  # contraction chunks for x @ W1 (8)
KC2 = H // 128  # contraction chunks for h @ W2 (2)
F32 = mybir.dt.float32
BF16 = mybir.dt.bfloat16

# Global tile order: (slot, t0, tn).  128-wide first (fast PE start via a
# small first DMA), 128-wide last (short drain tail).
TILES = [(0, 0, 128), (0, 128, 384), (1, 0, 384), (1, 384, 128)]
XCOLS = KC1 * 2 * CAP   # 8192 packed x / out columns

_build_cache: dict[tuple, object] = {}
LAST_RESULTS = None  # BassKernelResults of the most recent run (for profiling)


def _build():
    key = ("v2",)
    if key in _build_cache:
        return _build_cache[key]

    nc = bacc.Bacc("TRN2", target_bir_lowering=False, debug=False)

    # x / out packed tile-major: block q holds KC1*tn (resp. KC1*tn) cols,
    # [p, k*tn + c] = x[tok c][128k + p].
    xg = nc.dram_tensor("xg", [128, XCOLS], BF16, kind="ExternalInput")
    # per-slot weight pack: [w1 (j,k,128) 2048 | w2 (j,i,128) 2048]
    wpack = nc.dram_tensor("wpack", [128, 2, 2 * KC1 * H], BF16,
                           kind="ExternalInput")
    # bias[p, s, j]     = b1[e, 128j + p]   (j in 0..1)
    # bias[p, s, 2 + i] = b2[e, 128i + p]   (i in 0..7)
    bias = nc.dram_tensor("bias", [128, 2, KC2 + KC1], F32,
                          kind="ExternalInput")
    outP = nc.dram_tensor("outP", [128, XCOLS], BF16, kind="ExternalOutput")

    with tile.TileContext(nc) as tc:
        with (
            tc.tile_pool(name="const", bufs=1) as cpool,
            tc.tile_pool(name="w1p", bufs=4) as w1pool,
            tc.tile_pool(name="w2p", bufs=2) as w2pool,
            tc.tile_pool(name="xp", bufs=4) as xpool,
            tc.tile_pool(name="hp", bufs=2) as hpool,
            tc.tile_pool(name="op", bufs=2) as opool,
            tc.tile_pool(name="ph", bufs=2, space="PSUM") as phpool,
            tc.tile_pool(name="po", bufs=4, space="PSUM") as popool,
        ):
            # ---- all input DMAs up front, ramp-critical ones first ----
            w1t = {}
            w2t = {}
            # slot 0, j0 weights then the tiny first x tile: the PE can
            # start on just these two transfers.
            w1t[0, 0] = w1pool.tile([128, KC1 * 128], BF16, tag="w1t", name="w1t")
            nc.sync.dma_start(w1t[0, 0][:], wpack[:, 0, 0:1024])

            xts = []
            xoffs = []
            xoff = 0
            for q, (s, t0, tn) in enumerate(TILES):
                xt = xpool.tile([128, KC1, tn], BF16, tag=f"xt{q}", name=f"xt{q}")
                nc.scalar.dma_start(
                    xt[:],
                    xg[:, xoff:xoff + KC1 * tn].rearrange(
                        "p (k c) -> p k c", k=KC1))
                xts.append(xt)
                xoffs.append(xoff)
                xoff += KC1 * tn
                if q == 0:
                    # rest of slot-0 weights before the wide x tiles land
                    w1t[0, 1] = w1pool.tile([128, KC1 * 128], BF16, tag="w1t", name="w1t")
                    nc.sync.dma_start(w1t[0, 1][:], wpack[:, 0, 1024:2048])
                    w2t[0] = w2pool.tile([128, KC2 * D], BF16, tag="w2t", name="w2t")
                    nc.sync.dma_start(w2t[0][:], wpack[:, 0, 2048:4096])
                elif q == 1:
                    for j in range(KC2):
                        w1t[1, j] = w1pool.tile([128, KC1 * 128], BF16,
                                                tag="w1t", name="w1t")
                        nc.sync.dma_start(
                            w1t[1, j][:],
                            wpack[:, 1, 1024 * j:1024 * (j + 1)])
                    w2t[1] = w2pool.tile([128, KC2 * D], BF16, tag="w2t", name="w2t")
                    nc.sync.dma_start(w2t[1][:], wpack[:, 1, 2048:4096])

            bias_t = cpool.tile([128, 2, KC2 + KC1], F32)
            nc.scalar.dma_start(bias_t[:], bias[:])

            # ---- compute ----
            for q, (s, t0, tn) in enumerate(TILES):
                xt = xts[q]
                last = q == len(TILES) - 1

                ht = hpool.tile([128, KC2, tn], BF16, tag="ht")
                for j in range(KC2):
                    ph = phpool.tile([128, tn], F32, tag="ph")
                    for k in range(KC1):
                        nc.tensor.matmul(
                            ph[:], w1t[s, j][:, 128 * k:128 * (k + 1)],
                            xt[:, k, :],
                            start=(k == 0), stop=(k == KC1 - 1))
                    nc.scalar.activation(
                        ht[:, j, :], ph[:],
                        mybir.ActivationFunctionType.Relu,
                        bias=bias_t[:, s, j:j + 1])

                ot = opool.tile([128, KC1, tn], BF16, tag="ot")
                for i in range(KC1):
                    po = popool.tile([128, tn], F32, tag="po")
                    for j in range(KC2):
                        nc.tensor.matmul(
                            po[:],
                            w2t[s][:, 1024 * j + 128 * i:1024 * j + 128 * (i + 1)],
                            ht[:, j, :],
                            start=(j == 0), stop=(j == KC2 - 1))
                    bcol = bias_t[:, s, KC2 + i:KC2 + i + 1]
                    # GpSimd can't read PSUM; split copies Scalar/Vector
                    if i % 3 == 1:
                        nc.scalar.activation(
                            ot[:, i, :], po[:],
                            mybir.ActivationFunctionType.Identity,
                            bias=bcol)
                    else:
                        nc.vector.tensor_scalar_add(ot[:, i, :], po[:], bcol)
                    if last and i == KC1 // 2 - 1:
                        nc.sync.dma_start(
                            outP[:, xoffs[q]:xoffs[q] + (KC1 // 2) * tn
                                 ].rearrange("p (k c) -> p k c", k=KC1 // 2),
                            ot[:, :KC1 // 2, :])
                if last:
                    nc.sync.dma_start(
                        outP[:, xoffs[q] + (KC1 // 2) * tn:
                             xoffs[q] + KC1 * tn].rearrange(
                            "p (k c) -> p k c", k=KC1 - KC1 // 2),
                        ot[:, KC1 // 2:, :])
                else:
                    nc.sync.dma_start(
                        outP[:, xoffs[q]:xoffs[q] + KC1 * tn].rearrange(
                            "p (k c) -> p k c", k=KC1),
                        ot[:])

    nc.compile()
    _build_cache[key] = nc
    return nc


def kernel(x, y_idx, y, z, W1, b1, W2, b2):
    x = np.ascontiguousarray(np.asarray(x, dtype=np.float32))
    z = np.asarray(z, dtype=np.float32)
    W1 = np.asarray(W1, dtype=np.float32)
    b1 = np.asarray(b1, dtype=np.float32)
    W2 = np.asarray(W2, dtype=np.float32)
    b2 = np.asarray(b2, dtype=np.float32)
    e = np.asarray(y_idx).reshape(-1).astype(np.int64)
    B = x.shape[0]

    idxs = [np.flatnonzero(e == k) for k in range(NB)]

    nc = _build()

    in_maps = []
    for c in range(NCORES):
        xg = np.zeros((128, XCOLS), ml_dtypes.bfloat16)
        wpack = np.empty((128, 2, 2 * KC1 * H), ml_dtypes.bfloat16)
        bias = np.empty((128, 2, KC2 + KC1), np.float32)
        for s in range(2):
            k = 2 * c + s
            # w1 cols j*1024 + kk*128 + m = W1[k][128kk + p, 128j + m]
            wpack[:, s, :2048] = W1[k].reshape(
                KC1, 128, KC2, 128).transpose(1, 2, 0, 3).reshape(
                128, 2048).astype(ml_dtypes.bfloat16)
            # w2 cols 2048 + j*1024 + i*128 + m = W2[k][128j + p, 128i + m]
            wpack[:, s, 2048:] = W2[k].reshape(
                KC2, 128, KC1, 128).transpose(1, 0, 2, 3).reshape(
                128, 2048).astype(ml_dtypes.bfloat16)
            bias[:, s, :KC2] = b1[k].reshape(KC2, 128).T
            bias[:, s, KC2:] = b2[k].reshape(KC1, 128).T
        xoff = 0
        for s, t0, tn in TILES:
            k = 2 * c + s
            seg = idxs[k][t0:t0 + tn]
            n = len(seg)
            if n:
                full = np.zeros((128, KC1, tn), ml_dtypes.bfloat16)
                full[:, :, :n] = x[seg].reshape(
                    n, KC1, 128).transpose(2, 1, 0).astype(ml_dtypes.bfloat16)
                xg[:, xoff:xoff + KC1 * tn] = full.reshape(128, KC1 * tn)
            xoff += KC1 * tn
        in_maps.append({"xg": xg, "wpack": wpack, "bias": bias})

    res = run_bass_kernel_spmd(nc, in_maps, core_ids=list(range(NCORES)))
    global LAST_RESULTS
    LAST_RESULTS = res

    out = np.empty((B, D), np.float32)
    for c in range(NCORES):
        outP = res.results[c]["outP"]
        xoff = 0
        for s, t0, tn in TILES:
            k = 2 * c + s
            seg = idxs[k][t0:t0 + tn]
            n = len(seg)
            if n:
                blk = outP[:, xoff:xoff + KC1 * tn].reshape(128, KC1, tn)
                # blk[p, i, c] = o[token c, 128i + p]
                rows = blk[:, :, :n].transpose(2, 1, 0).reshape(
                    n, D).astype(np.float32)
                out[seg] = x[seg] + z[seg, k][:, None] * rows
            xoff += KC1 * tn

    # Overflow tokens beyond the per-expert device capacity: exact host
    # fp32 compute (~1% of the batch for a uniform router).
    for k in range(NB):
        seg = idxs[k][CAP:]
        if len(seg) == 0:
            continue
        h = np.maximum(x[seg] @ W1[k] + b1[k], 0.0)
        o = h @ W2[k] + b2[k]
        out[seg] = x[seg] + z[seg, k][:, None] * o
    return out


# revision 8
# speedup vs baseline: 1.2059x; 1.2059x over previous
"""Trainium2 Bass kernel for nn_DecSwitchedFC (MoE hard routing).

Math (per token b, expert e = y_idx[b]):
    out[b] = x[b] + z[b, e] * (relu(x[b] @ W1[e] + b1[e]) @ W2[e] + b2[e])

Strategy: expert-parallel over 8 NeuronCores, 2 experts per core, with a
fixed per-expert device capacity of 512 tokens (one expert = one token
column block).  Tokens beyond an expert's capacity (~1% of the batch for
a uniform router) are computed exactly on the host in fp32.  This gives
each core exactly 1024 device tokens with zero load imbalance across the
SPMD program.

Device data layout is fully "feature-major" (d or h on partitions, tokens
on the free axis) so no on-device transposes are needed:
    h^T[256, n]  = W1[e]^T(lhsT) @ x^T          (K=1024, 8 chunks)
    o^T[1024, n] = W2[e]^T(lhsT) @ relu(h^T+b1) (K=256, 2 chunks)

All matmuls run in bf16 (~2.4e-3 final rel err); o^T ships back in bf16
and the z-scale + residual are applied on the host in exact fp32.

Schedule notes (from perfetto traces):
  - DMA issue (~0.65us each on the issuing queue) and HBM bandwidth
    (~358 GB/s/core) are the binding constraints; total traffic is
    ~6.3 MB/core (x 2.1, W 2.1, out 2.1) ≈ 17.6us.
  - All DMAs go on the single Sync queue in exact consumption order
    (bias, w1-s0, x-t0, x-t1, w2-s0, x-t2, w1-s1, w2-s1, x-t3, outs):
    the 16 DMA engines stripe each transfer at ~395 GB/s aggregate, a
    queue is strict FIFO, and splitting across queues just splits the
    bandwidth.  (GpSimd DMA is a slow software path — 1.2us drains.)
  - The first tile is 128 tokens and only needs W1-j0 (256 KB) + x-t0
    (256 KB) before the PE can start; the last tile is 128 tokens with a
    split output DMA so the drain tail is short.
  - FC1 relu+bias runs on Scalar; the 8 FC2 bias-add copies per tile
    rotate across Vector/Scalar/GpSimd so no engine bottlenecks.
"""

import ml_dtypes
import numpy as np

import concourse.bacc as bacc
import concourse.mybir as mybir
import concourse.tile as tile
from concourse.bass_utils import run_bass_kernel_spmd

D = 1024        # model dim
H = 256         # bottleneck dim
NB = 16         # n experts
NCORES = 8
CAP = 512       # device tokens per expert (one 512-token column block)
KC1 = D // 128  # contraction chunks for x @ W1 (8)
KC2 = H // 128  # contraction chunks for h @ W2 (2)
F32 = mybir.dt.float32
BF16 = mybir.dt.bfloat16

# Global tile order: (slot, t0, tn).  128-wide first (fast PE start via a
# small first DMA), 128-wide last (short drain tail).
TILES = [(0, 0, 128), (0, 128, 384), (1, 0, 384), (1, 384, 128)]
XCOLS = KC1 * 2 * CAP   # 8192 packed x / out columns

_build_cache: dict[tuple, object] = {}
LAST_RESULTS = None  # BassKernelResults of the most recent run (for profiling)


def _build():
    key = ("v2",)
    if key in _build_cache:
        return _build_cache[key]

    nc = bacc.Bacc("TRN2", target_bir_lowering=False, debug=False)

    # x / out packed tile-major: block q holds KC1*tn (resp. KC1*tn) cols,
    # [p, k*tn + c] = x[tok c][128k + p].
    xg = nc.dram_tensor("xg", [128, XCOLS], BF16, kind="ExternalInput")
    # per-slot weight pack: [w1 (j,k,128) 2048 | w2 (j,i,128) 2048]
    wpack = nc.dram_tensor("wpack", [128, 2, 2 * KC1 * H], BF16,
                           kind="ExternalInput")
    # bias[p, s, j]     = b1[e, 128j + p]   (j in 0..1)
    # bias[p, s, 2 + i] = b2[e, 128i + p]   (i in 0..7)
    bias = nc.dram_tensor("bias", [128, 2, KC2 + KC1], F32,
                          kind="ExternalInput")
    outP = nc.dram_tensor("outP", [128, XCOLS], BF16, kind="ExternalOutput")

    with tile.TileContext(nc) as tc:
        with (
            tc.tile_pool(name="const", bufs=1) as cpool,
            tc.tile_pool(name="w1p", bufs=2) as w1pool,
            tc.tile_pool(name="w2p", bufs=2) as w2pool,
            tc.tile_pool(name="xp", bufs=4) as xpool,
            tc.tile_pool(name="hp", bufs=2) as hpool,
            tc.tile_pool(name="op", bufs=2) as opool,
            tc.tile_pool(name="ph", bufs=2, space="PSUM") as phpool,
            tc.tile_pool(name="po", bufs=4, space="PSUM") as popool,
        ):
            # ---- all input DMAs up front, one queue (Sync), in exact
            # consumption order: the 16 DMA engines stripe each transfer
            # at ~395 GB/s aggregate and a single queue is strict FIFO,
            # so this order IS the transfer schedule.
            bias_t = cpool.tile([128, 2, KC2 + KC1], F32)
            nc.sync.dma_start(bias_t[:], bias[:])

            w1t = {}
            w2t = {}
            w1t[0] = w1pool.tile([128, 2 * KC1 * 128], BF16, tag="w1t",
                                 name="w1t")
            nc.sync.dma_start(w1t[0][:], wpack[:, 0, 0:2048])

            xts = []
            xoffs = []

            def load_x(q, tn, xoff):
                xt = xpool.tile([128, KC1, tn], BF16, tag=f"xt{q}",
                                name=f"xt{q}")
                nc.sync.dma_start(
                    xt[:],
                    xg[:, xoff:xoff + KC1 * tn].rearrange(
                        "p (k c) -> p k c", k=KC1))
                xts.append(xt)
                xoffs.append(xoff)

            load_x(0, 128, 0)
            load_x(1, 384, KC1 * 128)
            w2t[0] = w2pool.tile([128, KC2 * D], BF16, tag="w2t", name="w2t")
            nc.sync.dma_start(w2t[0][:], wpack[:, 0, 2048:4096])
            load_x(2, 384, KC1 * 512)
            w1t[1] = w1pool.tile([128, 2 * KC1 * 128], BF16, tag="w1t",
                                 name="w1t")
            nc.sync.dma_start(w1t[1][:], wpack[:, 1, 0:2048])
            w2t[1] = w2pool.tile([128, KC2 * D], BF16, tag="w2t", name="w2t")
            nc.sync.dma_start(w2t[1][:], wpack[:, 1, 2048:4096])
            load_x(3, 128, KC1 * 896)

            # ---- compute ----
            for q, (s, t0, tn) in enumerate(TILES):
                xt = xts[q]
                last = q == len(TILES) - 1

                ht = hpool.tile([128, KC2, tn], BF16, tag="ht")
                for j in range(KC2):
                    ph = phpool.tile([128, tn], F32, tag="ph")
                    for k in range(KC1):
                        nc.tensor.matmul(
                            ph[:],
                            w1t[s][:, 1024 * j + 128 * k:1024 * j + 128 * (k + 1)],
                            xt[:, k, :],
                            start=(k == 0), stop=(k == KC1 - 1))
                    nc.scalar.activation(
                        ht[:, j, :], ph[:],
                        mybir.ActivationFunctionType.Relu,
                        bias=bias_t[:, s, j:j + 1])

                ot = opool.tile([128, KC1, tn], BF16, tag="ot")
                for i in range(KC1):
                    po = popool.tile([128, tn], F32, tag="po")
                    for j in range(KC2):
                        nc.tensor.matmul(
                            po[:],
                            w2t[s][:, 1024 * j + 128 * i:1024 * j + 128 * (i + 1)],
                            ht[:, j, :],
                            start=(j == 0), stop=(j == KC2 - 1))
                    bcol = bias_t[:, s, KC2 + i:KC2 + i + 1]
                    # GpSimd can't read PSUM; split copies Scalar/Vector
                    if i % 3 == 1:
                        nc.scalar.activation(
                            ot[:, i, :], po[:],
                            mybir.ActivationFunctionType.Identity,
                            bias=bcol)
                    else:
                        nc.vector.tensor_scalar_add(ot[:, i, :], po[:], bcol)
                    if last and i == KC1 // 2 - 1:
                        nc.sync.dma_start(
                            outP[:, xoffs[q]:xoffs[q] + (KC1 // 2) * tn
                                 ].rearrange("p (k c) -> p k c", k=KC1 // 2),
                            ot[:, :KC1 // 2, :])
                if last:
                    nc.sync.dma_start(
                        outP[:, xoffs[q] + (KC1 // 2) * tn:
                             xoffs[q] + KC1 * tn].rearrange(
                            "p (k c) -> p k c", k=KC1 - KC1 // 2),
                        ot[:, KC1 // 2:, :])
                else:
                    nc.sync.dma_start(
                        outP[:, xoffs[q]:xoffs[q] + KC1 * tn].rearrange(
                            "p (k c) -> p k c", k=KC1),
                        ot[:])

    nc.compile()
    _build_cache[key] = nc
    return nc


def kernel(x, y_idx, y, z, W1, b1, W2, b2):
    x = np.ascontiguousarray(np.asarray(x, dtype=np.float32))
    z = np.asarray(z, dtype=np.float32)
    W1 = np.asarray(W1, dtype=np.float32)
    b1 = np.asarray(b1, dtype=np.float32)
    W2 = np.asarray(W2, dtype=np.float32)
    b2 = np.asarray(b2, dtype=np.float32)
    e = np.asarray(y_idx).reshape(-1).astype(np.int64)
    B = x.shape[0]

    idxs = [np.flatnonzero(e == k) for k in range(NB)]

    nc = _build()

    in_maps = []
    for c in range(NCORES):
        xg = np.zeros((128, XCOLS), ml_dtypes.bfloat16)
        wpack = np.empty((128, 2, 2 * KC1 * H), ml_dtypes.bfloat16)
        bias = np.empty((128, 2, KC2 + KC1), np.float32)
        for s in range(2):
            k = 2 * c + s
            # w1 cols j*1024 + kk*128 + m = W1[k][128kk + p, 128j + m]
            wpack[:, s, :2048] = W1[k].reshape(
                KC1, 128, KC2, 128).transpose(1, 2, 0, 3).reshape(
                128, 2048).astype(ml_dtypes.bfloat16)
            # w2 cols 2048 + j*1024 + i*128 + m = W2[k][128j + p, 128i + m]
            wpack[:, s, 2048:] = W2[k].reshape(
                KC2, 128, KC1, 128).transpose(1, 0, 2, 3).reshape(
                128, 2048).astype(ml_dtypes.bfloat16)
            bias[:, s, :KC2] = b1[k].reshape(KC2, 128).T
            bias[:, s, KC2:] = b2[k].reshape(KC1, 128).T
        xoff = 0
        for s, t0, tn in TILES:
            k = 2 * c + s
            seg = idxs[k][t0:t0 + tn]
            n = len(seg)
            if n:
                full = np.zeros((128, KC1, tn), ml_dtypes.bfloat16)
                full[:, :, :n] = x[seg].reshape(
                    n, KC1, 128).transpose(2, 1, 0).astype(ml_dtypes.bfloat16)
                xg[:, xoff:xoff + KC1 * tn] = full.reshape(128, KC1 * tn)
            xoff += KC1 * tn
        in_maps.append({"xg": xg, "wpack": wpack, "bias": bias})

    res = run_bass_kernel_spmd(nc, in_maps, core_ids=list(range(NCORES)))
    global LAST_RESULTS
    LAST_RESULTS = res

    out = np.empty((B, D), np.float32)
    for c in range(NCORES):
        outP = res.results[c]["outP"]
        xoff = 0
        for s, t0, tn in TILES:
            k = 2 * c + s
            seg = idxs[k][t0:t0 + tn]
            n = len(seg)
            if n:
                blk = outP[:, xoff:xoff + KC1 * tn].reshape(128, KC1, tn)
                # blk[p, i, c] = o[token c, 128i + p]
                rows = blk[:, :, :n].transpose(2, 1, 0).reshape(
                    n, D).astype(np.float32)
                out[seg] = x[seg] + z[seg, k][:, None] * rows
            xoff += KC1 * tn

    # Overflow tokens beyond the per-expert device capacity: exact host
    # fp32 compute (~1% of the batch for a uniform router).
    for k in range(NB):
        seg = idxs[k][CAP:]
        if len(seg) == 0:
            continue
        h = np.maximum(x[seg] @ W1[k] + b1[k], 0.0)
        o = h @ W2[k] + b2[k]
        out[seg] = x[seg] + z[seg, k][:, None] * o
    return out
